# revision 15
# baseline (speedup 1.0000x reference)
"""Trainium2 Bass kernel for CausalGNNLayer (per-node-type Linear, MoE-style routing).

Semantics (matching the reference):
    out[n, :] = x[n, :] @ W[node_types[n]].T + b[node_types[n]]
edge_index is unused by the op.

Strategy:
- Host-side routing-aware sharding: stable-sort nodes by type, split each
  type's node list into two halves -> 8 groups (4 types x 2 cores).
- bf16 operands with fp32 PSUM accumulation; bf16 output storage.  This
  halves both HBM directions vs fp32 and keeps rel-err ~3e-3.
- Weight-stationary matmul schedule (out.T layout): psum[o_blk, nodes] +=
  w[k, o_blk].T @ xT[k, nodes].  The stationary operand (a 128x128 tile of
  W.T) is reused across all chunks of a group, so LDWEIGHTS amortizes and
  hides under the moving stream; matmuls run back-to-back at the tensor
  engine's streaming rate (~N/2.4GHz).
- Variable chunk grid: a 128-node mini chunk first (compute starts after
  ~130KB of DMA), 512-node chunks in graded-size stationary groups (so the
  x DMA stream stays ahead during the startup transient), and a small final
  chunk sized so P just covers the largest shard (minimizes padded FLOPs).
- PE p-state warmup: dummy LDWEIGHTS keep the tensor engine busy from
  sequencer start so it ramps toward full clock before real matmuls.
- Drain (bias add + fp32->bf16 downcast) alternates between the Vector and
  the otherwise-idle Scalar engine; in out.T layout the bias is a
  per-partition scalar, which both engines support natively.
- Host scatters the 8 bf16 output shards back into the full [N, 512] fp32
  output.
"""

import numpy as np
import ml_dtypes
from contextlib import ExitStack

import concourse.bass as bass
import concourse.mybir as mybir
import concourse.tile as tile
from concourse.bass_utils import run_bass_kernel_spmd

N_CORES = 8
IN_CH = 512
OUT_CH = 512
NUM_TYPES = 4
P_BLK = 128          # partition count
KT = IN_CH // P_BLK  # 4 contraction tiles
CHUNK_N = 512        # main chunk width == psum bank capacity (fp32)
MINI_N = 128         # first chunk width (early compute start)
XBUFS = 10           # x-chunk prefetch depth
PSBUFS = 2           # psum mega-tile ring (2 x 4 banks = all 8)
OBUFS = 4            # output staging depth
WARMUP_LDW = 20      # dummy LDWEIGHTS to ramp the PE p-state during DMA wait

# Set by test harness to capture HW profile; kernel works without it.
TRACE = False
LAST_RESULTS = None

_compile_cache: dict = {}

_legal_nop_counter = [0]


def _legalize_waits(nc: bass.Bass) -> None:
    """This walrus codegen only encodes ONE sync wait per engine instruction.
    Tile's scheduler attaches several.  Split: hoist all-but-one wait of any
    multi-wait instruction into preceding same-engine NoOps (one wait each) —
    semantically identical (the engine stalls on each wait in program order)."""
    for fn in nc.m.functions:
        for blk in fn.blocks:
            insts = blk.instructions
            out = []
            changed = False
            for inst in insts:
                si = inst.sync_info
                waits = list(si.on_wait) if si is not None and si.on_wait else []
                if len(waits) > 1:
                    changed = True
                    for w in waits[:-1]:
                        _legal_nop_counter[0] += 1
                        nop = mybir.InstNoOp(
                            name=f"waitsplit-{_legal_nop_counter[0]}",
                            ins=[],
                            outs=[],
                            engine=inst.engine,
                        )
                        nop.sync_info = mybir.SyncInfo(on_wait=[w], on_update=[])
                        out.append(nop)
                    inst.sync_info = mybir.SyncInfo(
                        on_wait=[waits[-1]], on_update=list(si.on_update or [])
                    )
                out.append(inst)
            if changed:
                blk.instructions = out


def _plan(P_needed: int):
    """Chunk widths [MINI_N, 512, ..., 512, final] covering >= P_needed,
    and stationary-reuse groups (graded sizes over the 512-chunks)."""
    rem = P_needed - MINI_N
    n512 = max(0, (rem - 1) // CHUNK_N)
    final = rem - n512 * CHUNK_N
    final = ((final + 63) // 64) * 64  # pad to 64 for sane DMA runs
    widths = [MINI_N] + [CHUNK_N] * n512 + ([final] if final else [])
    # groups: mini alone; graded sizes over the 512s; final alone
    groups = [[0]]
    graded = [1, 1, 2, 3]
    idx = 1
    gi = 0
    while idx < 1 + n512:
        s = graded[gi] if gi < len(graded) else 4
        s = min(s, 1 + n512 - idx)
        groups.append(list(range(idx, idx + s)))
        idx += s
        gi += 1
    if final:
        groups.append([1 + n512])
    offs = np.concatenate([[0], np.cumsum(widths)]).astype(int)
    return widths, list(offs[:-1]), int(offs[-1]), groups


def _build_bass(plan_key) -> bass.Bass:
    widths, offs, P, groups = plan_key
    nc = bass.Bass("TRN2")
    f32 = mybir.dt.float32
    bf16 = mybir.dt.bfloat16

    OBLKS = OUT_CH // P_BLK  # 4
    nchunks = len(widths)

    # x stored chunk-major, each chunk a [128, KT, 512] block (tail columns
    # of narrow chunks unused); slicing [:, :, :w] keeps 512-wide chunk DMAs
    # as single 4KB contiguous runs per partition.
    xT = nc.dram_tensor(
        "xT", [nchunks, P_BLK, KT, CHUNK_N], bf16, kind="ExternalInput"
    )
    w = nc.dram_tensor("w", [IN_CH, OUT_CH], bf16, kind="ExternalInput")
    # bias2[p, oblk] = b[oblk*128 + p]
    bias2 = nc.dram_tensor("bias2", [P_BLK, OBLKS], f32, kind="ExternalInput")
    # outT[oblk, p, n] = out[n, oblk*128+p]
    out = nc.dram_tensor("out", [OBLKS, P_BLK, P], bf16, kind="ExternalOutput")

    w_v = w.ap().rearrange("(k p) o -> p k o", p=P_BLK)

    with ExitStack() as ctx:
        tc = ctx.enter_context(tile.TileContext(nc))
        wp = ctx.enter_context(tc.tile_pool(name="w", bufs=1))
        warmp = ctx.enter_context(tc.tile_pool(name="warm", bufs=1))
        actp = ctx.enter_context(tc.tile_pool(name="actw", bufs=1))
        bp = ctx.enter_context(tc.tile_pool(name="b", bufs=1))
        xp = ctx.enter_context(tc.tile_pool(name="x", bufs=XBUFS))
        pp = ctx.enter_context(tc.tile_pool(name="ps", bufs=PSBUFS, space="PSUM"))
        op = ctx.enter_context(tc.tile_pool(name="o", bufs=OBUFS))

        # PE p-state warmup: the tensor engine ramps 0.65->1.2->2.4 GHz with
        # continuous busy time; dummy weight loads keep it busy while the
        # first data DMAs land.
        warm_sb = warmp.tile([P_BLK, P_BLK], bf16)
        nc.gpsimd.memset(warm_sb[:], 0)
        for _ in range(WARMUP_LDW):
            nc.tensor.ldweights(warm_sb[:])
        # ACT-table preload (~1.3us one-time) so the first real drain on the
        # scalar engine isn't delayed; separate tile so it can't serialize
        # ahead of the LDWEIGHTS warmup.
        act_sb = actp.tile([1, 1], f32)
        nc.gpsimd.memset(act_sb[:], 0)
        nc.scalar.add(act_sb[:], act_sb[:], 0.0)

        x_tiles: dict[int, object] = {}

        def fetch_chunk(c: int):
            if c not in x_tiles:
                wd = widths[c]
                t = xp.tile([P_BLK, KT, wd], bf16, tag="x")
                nc.sync.dma_start(t[:], xT.ap()[c][:, :, 0:wd])
                x_tiles[c] = t

        w_sb = wp.tile([P_BLK, KT, OUT_CH], bf16)
        # issue order: mini chunk 0, w k0, rest of w — first matmul can start
        # after ~260KB of DMA.
        fetch_chunk(0)
        for k in range(KT):
            nc.sync.dma_start(w_sb[:, k, :], w_v[:, k, :])
        b_sb = bp.tile([P_BLK, OBLKS], f32)
        nc.sync.dma_start(b_sb[:], bias2.ap())

        drain_flip = [0]
        for gi, grp in enumerate(groups):
            for c in grp:
                fetch_chunk(c)
            if gi + 1 < len(groups):
                for c in groups[gi + 1]:
                    fetch_chunk(c)
            gw = sum(widths[c] for c in grp)
            goff = offs[grp[0]]
            for oblk in range(OBLKS):
                ps = pp.tile([P_BLK, gw], f32, tag="ps")
                for k in range(KT):
                    lhsT = w_sb[:, k, oblk * P_BLK : (oblk + 1) * P_BLK]
                    loc = 0
                    for c in grp:
                        nc.tensor.matmul(
                            ps[:, loc : loc + widths[c]],
                            lhsT=lhsT,
                            rhs=x_tiles[c][:, k, :],
                            start=(k == 0),
                            stop=(k == KT - 1),
                        )
                        loc += widths[c]
                o_sb = op.tile([P_BLK, gw], bf16, tag="o")
                bias_ap = b_sb[:, oblk : oblk + 1]
                if drain_flip[0] % 2 == 0:
                    nc.vector.tensor_scalar_add(o_sb[:], ps[:], bias_ap)
                else:
                    nc.scalar.add(o_sb[:], ps[:], bias_ap)
                drain_flip[0] += 1
                nc.sync.dma_start(out.ap()[oblk, :, goff : goff + gw], o_sb[:])
    _legalize_waits(nc)
    return nc


def _get_compiled(plan_key) -> bass.Bass:
    key = (tuple(plan_key[0]), plan_key[2])
    if key not in _compile_cache:
        _compile_cache[key] = _build_bass(plan_key)
    return _compile_cache[key]


def kernel(x, edge_index, node_types, W, b):
    global LAST_RESULTS
    x = np.asarray(x, dtype=np.float32)
    nt = np.asarray(node_types).astype(np.int64)
    W = np.asarray(W, dtype=np.float32)
    b = np.asarray(b, dtype=np.float32)
    N = x.shape[0]

    # Route nodes: stable sort by type, split each type across 2 cores.
    order = np.argsort(nt, kind="stable")
    counts = np.bincount(nt, minlength=NUM_TYPES)
    shards = []
    start = 0
    for t in range(NUM_TYPES):
        c = int(counts[t])
        idx = order[start : start + c]
        start += c
        h = (c + 1) // 2
        shards.append(idx[:h])
        shards.append(idx[h:])

    P_needed = max(1, max(len(g) for g in shards))
    plan = _plan(P_needed)
    widths, offs, P, groups = plan
    nchunks = len(widths)

    nc = _get_compiled(plan)

    in_maps = []
    for gi, g in enumerate(shards):
        t = gi // 2
        xs = np.zeros((P, IN_CH), np.float32)
        if len(g):
            xs[: len(g)] = x[g]
        xsT = xs.T.astype(ml_dtypes.bfloat16)  # [512, P]
        xbuf = np.zeros((nchunks, P_BLK, KT, CHUNK_N), ml_dtypes.bfloat16)
        for c in range(nchunks):
            wd = widths[c]
            seg = xsT[:, offs[c] : offs[c] + wd].reshape(KT, P_BLK, wd)
            xbuf[c, :, :, :wd] = seg.transpose(1, 0, 2)
        in_maps.append(
            {
                "xT": xbuf,
                "w": np.ascontiguousarray(W[t].T).astype(ml_dtypes.bfloat16),
                "bias2": np.ascontiguousarray(
                    b[t].reshape(4, P_BLK).T.astype(np.float32)
                ),
            }
        )

    res = run_bass_kernel_spmd(nc, in_maps, list(range(N_CORES)), trace=TRACE)
    LAST_RESULTS = res

    out = np.empty((N, OUT_CH), np.float32)
    for gi, g in enumerate(shards):
        if len(g):
            # outT [4, 128, P] -> [P, 512] node-major
            o = res.results[gi]["out"].reshape(OUT_CH, P).T.astype(np.float32)
            out[g] = o[: len(g)]
    return out


# revision 18
# speedup vs baseline: 1.0188x; 1.0188x over previous
"""Trainium2 Bass kernel for CausalGNNLayer (per-node-type Linear, MoE-style routing).

Semantics (matching the reference):
    out[n, :] = x[n, :] @ W[node_types[n]].T + b[node_types[n]]
edge_index is unused by the op.

Strategy:
- Host-side routing-aware sharding: stable-sort nodes by type, split each
  type's node list into two halves -> 8 groups (4 types x 2 cores).
- bf16 operands with fp32 PSUM accumulation; bf16 output storage.  This
  halves both HBM directions vs fp32 and keeps rel-err ~3e-3.
- Weight-stationary matmul schedule (out.T layout): psum[o_blk, nodes] +=
  w[k, o_blk].T @ xT[k, nodes].  The stationary operand (a 128x128 tile of
  W.T) is reused across all chunks of a group, so LDWEIGHTS amortizes and
  hides under the moving stream; matmuls run back-to-back at the tensor
  engine's streaming rate (~N/2.4GHz).
- Variable chunk grid: a 128-node mini chunk first (compute starts after
  ~130KB of DMA), 512-node chunks in graded-size stationary groups (so the
  x DMA stream stays ahead during the startup transient), and a small final
  chunk sized so P just covers the largest shard (minimizes padded FLOPs).
- PE p-state warmup: dummy LDWEIGHTS keep the tensor engine busy from
  sequencer start so it ramps toward full clock before real matmuls.
- Drain (bias add + fp32->bf16 downcast) alternates between the Vector and
  the otherwise-idle Scalar engine; in out.T layout the bias is a
  per-partition scalar, which both engines support natively.
- Host scatters the 8 bf16 output shards back into the full [N, 512] fp32
  output.
"""

import numpy as np
import ml_dtypes
from contextlib import ExitStack

import concourse.bass as bass
import concourse.mybir as mybir
import concourse.tile as tile
from concourse.bass_utils import run_bass_kernel_spmd

N_CORES = 8
IN_CH = 512
OUT_CH = 512
NUM_TYPES = 4
P_BLK = 128          # partition count
KT = IN_CH // P_BLK  # 4 contraction tiles
CHUNK_N = 512        # main chunk width == psum bank capacity (fp32)
MINI_N = 128         # first chunk width (early compute start)
XBUFS = 10           # x-chunk prefetch depth
PSBUFS = 2           # psum mega-tile ring (2 x 4 banks = all 8)
OBUFS = 4            # output staging depth
WARMUP_LDW = 26      # dummy LDWEIGHTS to ramp the PE p-state during DMA wait

# Set by test harness to capture HW profile; kernel works without it.
TRACE = False
LAST_RESULTS = None

_compile_cache: dict = {}

_legal_nop_counter = [0]


def _legalize_waits(nc: bass.Bass) -> None:
    """This walrus codegen only encodes ONE sync wait per engine instruction.
    Tile's scheduler attaches several.  Split: hoist all-but-one wait of any
    multi-wait instruction into preceding same-engine NoOps (one wait each) —
    semantically identical (the engine stalls on each wait in program order)."""
    for fn in nc.m.functions:
        for blk in fn.blocks:
            insts = blk.instructions
            out = []
            changed = False
            for inst in insts:
                si = inst.sync_info
                waits = list(si.on_wait) if si is not None and si.on_wait else []
                if len(waits) > 1:
                    changed = True
                    for w in waits[:-1]:
                        _legal_nop_counter[0] += 1
                        nop = mybir.InstNoOp(
                            name=f"waitsplit-{_legal_nop_counter[0]}",
                            ins=[],
                            outs=[],
                            engine=inst.engine,
                        )
                        nop.sync_info = mybir.SyncInfo(on_wait=[w], on_update=[])
                        out.append(nop)
                    inst.sync_info = mybir.SyncInfo(
                        on_wait=[waits[-1]], on_update=list(si.on_update or [])
                    )
                out.append(inst)
            if changed:
                blk.instructions = out


def _plan(P_needed: int):
    """Chunk widths [MINI_N, 512, ..., 512, final] covering >= P_needed,
    and stationary-reuse groups (graded sizes over the 512-chunks)."""
    rem = P_needed - MINI_N
    n512 = max(0, (rem - 1) // CHUNK_N)
    final = rem - n512 * CHUNK_N
    final = ((final + 63) // 64) * 64  # pad to 64 for sane DMA runs
    widths = [MINI_N] + [CHUNK_N] * n512 + ([final] if final else [])
    # groups: mini alone; graded sizes over the 512s (so the x-DMA stream
    # stays ahead of each group's front-loaded chunk needs); final alone
    groups = [[0]]
    graded = [1, 2, 3]
    idx = 1
    gi = 0
    while idx < 1 + n512:
        s = graded[gi] if gi < len(graded) else 4
        s = min(s, 1 + n512 - idx)
        groups.append(list(range(idx, idx + s)))
        idx += s
        gi += 1
    if final:
        groups.append([1 + n512])
    offs = np.concatenate([[0], np.cumsum(widths)]).astype(int)
    return widths, list(offs[:-1]), int(offs[-1]), groups


def _build_bass(plan_key) -> bass.Bass:
    widths, offs, P, groups = plan_key
    nc = bass.Bass("TRN2")
    f32 = mybir.dt.float32
    bf16 = mybir.dt.bfloat16

    OBLKS = OUT_CH // P_BLK  # 4
    nchunks = len(widths)

    # x stored chunk-major, each chunk a [128, KT, 512] block (tail columns
    # of narrow chunks unused); slicing [:, :, :w] keeps 512-wide chunk DMAs
    # as single 4KB contiguous runs per partition.
    xT = nc.dram_tensor(
        "xT", [nchunks, P_BLK, KT, CHUNK_N], bf16, kind="ExternalInput"
    )
    w = nc.dram_tensor("w", [IN_CH, OUT_CH], bf16, kind="ExternalInput")
    # bias2[p, oblk] = b[oblk*128 + p]
    bias2 = nc.dram_tensor("bias2", [P_BLK, OBLKS], f32, kind="ExternalInput")
    # outT[oblk, p, n] = out[n, oblk*128+p]
    out = nc.dram_tensor("out", [OBLKS, P_BLK, P], bf16, kind="ExternalOutput")

    w_v = w.ap().rearrange("(k p) o -> p k o", p=P_BLK)

    with ExitStack() as ctx:
        tc = ctx.enter_context(tile.TileContext(nc))
        wp = ctx.enter_context(tc.tile_pool(name="w", bufs=1))
        warmp = ctx.enter_context(tc.tile_pool(name="warm", bufs=1))
        actp = ctx.enter_context(tc.tile_pool(name="actw", bufs=1))
        bp = ctx.enter_context(tc.tile_pool(name="b", bufs=1))
        xp = ctx.enter_context(tc.tile_pool(name="x", bufs=XBUFS))
        pp = ctx.enter_context(tc.tile_pool(name="ps", bufs=PSBUFS, space="PSUM"))
        op = ctx.enter_context(tc.tile_pool(name="o", bufs=OBUFS))

        # PE p-state warmup: the tensor engine ramps 0.65->1.2->2.4 GHz with
        # continuous busy time; dummy weight loads keep it busy while the
        # first data DMAs land.
        warm_sb = warmp.tile([P_BLK, P_BLK], bf16)
        nc.gpsimd.memset(warm_sb[:], 0)
        for _ in range(WARMUP_LDW):
            nc.tensor.ldweights(warm_sb[:])
        # ACT-table preload (~1.3us one-time) so the first real drain on the
        # scalar engine isn't delayed; separate tile so it can't serialize
        # ahead of the LDWEIGHTS warmup.
        act_sb = actp.tile([1, 1], f32)
        nc.gpsimd.memset(act_sb[:], 0)
        nc.scalar.add(act_sb[:], act_sb[:], 0.0)

        x_tiles: dict[int, object] = {}

        def fetch_chunk(c: int):
            if c not in x_tiles:
                wd = widths[c]
                t = xp.tile([P_BLK, KT, wd], bf16, tag="x")
                nc.sync.dma_start(t[:], xT.ap()[c][:, :, 0:wd])
                x_tiles[c] = t

        w_sb = wp.tile([P_BLK, KT, OUT_CH], bf16)
        # issue order: mini chunk 0, w k0, rest of w — first matmul can start
        # after ~260KB of DMA.
        fetch_chunk(0)
        for k in range(KT):
            nc.sync.dma_start(w_sb[:, k, :], w_v[:, k, :])
        b_sb = bp.tile([P_BLK, OBLKS], f32)
        nc.sync.dma_start(b_sb[:], bias2.ap())

        drain_flip = [0]
        for gi, grp in enumerate(groups):
            for c in grp:
                fetch_chunk(c)
            if gi + 1 < len(groups):
                for c in groups[gi + 1]:
                    fetch_chunk(c)
            gw = sum(widths[c] for c in grp)
            goff = offs[grp[0]]
            if len(grp) == 1:
                # Small group: all 4 oblks share one mega psum tile (one pool
                # allocation per group) so the 2-deep psum ring recycles at
                # group cadence, not oblk cadence — the drain turnaround
                # (~1.5us) otherwise stalls the PE on short oblk bursts.
                ps = pp.tile([P_BLK, OBLKS * CHUNK_N], f32, tag="ps")
                for oblk in range(OBLKS):
                    pslice = ps[:, oblk * CHUNK_N : oblk * CHUNK_N + gw]
                    for k in range(KT):
                        nc.tensor.matmul(
                            pslice,
                            lhsT=w_sb[:, k, oblk * P_BLK : (oblk + 1) * P_BLK],
                            rhs=x_tiles[grp[0]][:, k, :],
                            start=(k == 0),
                            stop=(k == KT - 1),
                        )
                for oblk in range(OBLKS):
                    pslice = ps[:, oblk * CHUNK_N : oblk * CHUNK_N + gw]
                    o_sb = op.tile([P_BLK, gw], bf16, tag="o")
                    bias_ap = b_sb[:, oblk : oblk + 1]
                    if drain_flip[0] % 2 == 0:
                        nc.vector.tensor_scalar_add(o_sb[:], pslice, bias_ap)
                    else:
                        nc.scalar.add(o_sb[:], pslice, bias_ap)
                    drain_flip[0] += 1
                    nc.sync.dma_start(
                        out.ap()[oblk, :, goff : goff + gw], o_sb[:]
                    )
                continue
            for oblk in range(OBLKS):
                ps = pp.tile([P_BLK, gw], f32, tag="ps")
                for k in range(KT):
                    lhsT = w_sb[:, k, oblk * P_BLK : (oblk + 1) * P_BLK]
                    loc = 0
                    for c in grp:
                        nc.tensor.matmul(
                            ps[:, loc : loc + widths[c]],
                            lhsT=lhsT,
                            rhs=x_tiles[c][:, k, :],
                            start=(k == 0),
                            stop=(k == KT - 1),
                        )
                        loc += widths[c]
                o_sb = op.tile([P_BLK, gw], bf16, tag="o")
                bias_ap = b_sb[:, oblk : oblk + 1]
                if drain_flip[0] % 2 == 0:
                    nc.vector.tensor_scalar_add(o_sb[:], ps[:], bias_ap)
                else:
                    nc.scalar.add(o_sb[:], ps[:], bias_ap)
                drain_flip[0] += 1
                nc.sync.dma_start(out.ap()[oblk, :, goff : goff + gw], o_sb[:])
    _legalize_waits(nc)
    return nc


def _get_compiled(plan_key) -> bass.Bass:
    key = (tuple(plan_key[0]), plan_key[2])
    if key not in _compile_cache:
        _compile_cache[key] = _build_bass(plan_key)
    return _compile_cache[key]


def kernel(x, edge_index, node_types, W, b):
    global LAST_RESULTS
    x = np.asarray(x, dtype=np.float32)
    nt = np.asarray(node_types).astype(np.int64)
    W = np.asarray(W, dtype=np.float32)
    b = np.asarray(b, dtype=np.float32)
    N = x.shape[0]

    # Route nodes: stable sort by type, split each type across 2 cores.
    order = np.argsort(nt, kind="stable")
    counts = np.bincount(nt, minlength=NUM_TYPES)
    shards = []
    start = 0
    for t in range(NUM_TYPES):
        c = int(counts[t])
        idx = order[start : start + c]
        start += c
        h = (c + 1) // 2
        shards.append(idx[:h])
        shards.append(idx[h:])

    P_needed = max(1, max(len(g) for g in shards))
    plan = _plan(P_needed)
    widths, offs, P, groups = plan
    nchunks = len(widths)

    nc = _get_compiled(plan)

    in_maps = []
    for gi, g in enumerate(shards):
        t = gi // 2
        xs = np.zeros((P, IN_CH), np.float32)
        if len(g):
            xs[: len(g)] = x[g]
        xsT = xs.T.astype(ml_dtypes.bfloat16)  # [512, P]
        xbuf = np.zeros((nchunks, P_BLK, KT, CHUNK_N), ml_dtypes.bfloat16)
        for c in range(nchunks):
            wd = widths[c]
            seg = xsT[:, offs[c] : offs[c] + wd].reshape(KT, P_BLK, wd)
            xbuf[c, :, :, :wd] = seg.transpose(1, 0, 2)
        in_maps.append(
            {
                "xT": xbuf,
                "w": np.ascontiguousarray(W[t].T).astype(ml_dtypes.bfloat16),
                "bias2": np.ascontiguousarray(
                    b[t].reshape(4, P_BLK).T.astype(np.float32)
                ),
            }
        )

    res = run_bass_kernel_spmd(nc, in_maps, list(range(N_CORES)), trace=TRACE)
    LAST_RESULTS = res

    out = np.empty((N, OUT_CH), np.float32)
    for gi, g in enumerate(shards):
        if len(g):
            # outT [4, 128, P] -> [P, 512] node-major
            o = res.results[gi]["out"].reshape(OUT_CH, P).T.astype(np.float32)
            out[g] = o[: len(g)]
    return out
